# revision 32
# baseline (speedup 1.0000x reference)
"""Trainium2 Bass kernel for nn_BCAblock_Anchor (bilateral window cross-attention).

Sharding: spatial over image rows. 8 cores x 24 rows each (both batches on
every core); k/v inputs carry a +-4 row halo (zero padded at image borders,
matching the reference's zero padding of k/v). No collectives.

Host<->device transport is the bottleneck in this environment (~60MB/s h2d,
~37MB/s d2h through the axon relay), so the run path is built around byte
minimization and call-overhead elimination:
  - the jitted shard_map executable is built once and cached (the stock
    run_bass_kernel_spmd/run_bass_via_pjrt path rebuilds jax.jit every call,
    which re-traces, re-compiles and re-loads the NEFF each time);
  - all bulk inputs travel as ONE bf16 blob per core (xt, haloed x0/x1,
    bf16 weight blocks) plus one small f32 blob (biases/LN params/cpb bias);
  - value-independent constants (identity, block-diag ones, 1/128 matrix)
    live in a device-cached array, transferred only on the first call;
  - the donated output buffer is generated on-device (no zeros upload);
  - the kernel returns delta = out - xt in bf16; the exact f32 residual
    add happens on the host.
"""

import sys

sys.path.insert(0, "/opt/trn_rl_repo")

from contextlib import ExitStack

import numpy as np
import ml_dtypes

import concourse.bass as bass
import concourse.bacc as bacc
import concourse.mybir as mybir
import concourse.tile as tile

F32 = mybir.dt.float32
BF16 = mybir.dt.bfloat16
I8 = mybir.dt.int8
AF = mybir.ActivationFunctionType
OP = mybir.AluOpType
NPBF16 = ml_dtypes.bfloat16

B, C, NH, WS = 2, 128, 4, 9
H, W, HC, MD = 192, 192, 32, 4
W2 = WS * WS                 # 81
NCORES = 8
RPC = H // NCORES            # 24 own rows per core
HR = RPC + 2 * MD            # 32 haloed rows per core
PW = W + 2 * MD              # 200 padded row width
NPIX = RPC * W               # 4608 own pixels per batch per core
NHPIX = HR * W               # 6144 haloed pixels per batch per core

SR = 12                      # rows per sub-tile pass
NST = RPC // SR              # 2 sub-tiles
SHR = SR + 2 * MD            # 20 haloed rows per pass
SNPIX = SR * W               # 2304
SNHPIX = SHR * W             # 3840
SSLAB = SHR * PW             # 4000
SNOWN = SR * PW              # 2400 own-window (incl x pads)
GUARD = 8
OWN0 = GUARD + MD * PW
CHSZ = 480
NCH = SNOWN // CHSZ          # 5

# int8 x tensors (rows of 128), one dram tensor each so the host can
# stream each upload as soon as its quantization finishes
RQH = B * NHPIX              # bq0/bq1: 12288 rows per core
RQT = B * NPIX               # bqt: 9216 rows per core

# bf16 blob: weight blocks only
WB0 = 0
NWB = 12
R16 = NWB * 128              # 1536 rows per core

# small f32 blob columns: [128, SMW]
SM_QW = 0                    # q_w as a [128,128] block
SM_BIAS = 128                # bias_d, 81 cols
SM_SC = 209                  # scale128
SM_QB, SM_KB, SM_VB, SM_PJB = 210, 211, 212, 213
SM_F1B = 214                 # 4 cols
SM_F2B = 218
SM_N1W, SM_N1B, SM_N2W, SM_N2B = 219, 220, 221, 222
SM_EPS24, SM_EPS6 = 223, 224
SC_X0 = 225                  # per-pixel dequant scales, 96 cols
SC_X1 = SC_X0 + B * NHPIX // 128
SC_XT = SC_X1 + B * NHPIX // 128   # 72 cols
SMW = SC_XT + B * NPIX // 128      # 489

# cached const f32 columns: e128 | j128 | eye_f32
CF_E128, CF_J128, CF_EYE = 0, 128, 256
CFW = 384


def _trace(ctx, tc, io):
    nc = tc.nc

    consts = ctx.enter_context(tc.tile_pool(name="consts", bufs=1))
    slabs = ctx.enter_context(tc.tile_pool(name="slabs", bufs=1))
    work = ctx.enter_context(tc.tile_pool(name="work", bufs=2))
    post = ctx.enter_context(tc.tile_pool(name="post", bufs=1))
    dloop = ctx.enter_context(tc.tile_pool(name="dloop", bufs=4))
    psum = ctx.enter_context(tc.tile_pool(name="psum", bufs=4, space="PSUM"))

    blob16 = io["blob16"]
    blobsm = io["blobsm"]
    constf = io["constf"]

    def cload_cols(src, c0, cols, shape, dtype=F32, tag=None):
        t = consts.tile(shape, dtype, tag=tag)
        nc.sync.dma_start(t[:], src[:, c0:c0 + cols])
        return t

    def cload_wblk(idx, tag, nblk=1):
        """Load nblk consecutive [128,128] bf16 weight blocks into one tile."""
        t = consts.tile([128, 128 * nblk], BF16, tag=tag)
        for i in range(nblk):
            r0 = WB0 + (idx + i) * 128
            nc.gpsimd.dma_start(t[:, i * 128:(i + 1) * 128],
                                blob16[r0:r0 + 128, :])
        return t

    eye16 = consts.tile([128, 128], BF16, tag="eye16")
    nc.gpsimd.dma_start(eye16[:], io["eye16c"][:])
    e128f = cload_cols(constf, CF_E128, 128, [128, 128], tag="e128")
    j128 = cload_cols(constf, CF_J128, 128, [128, 128], tag="j128")
    eyef = cload_cols(constf, CF_EYE, 128, [128, 128], tag="eyef")

    qw = cload_cols(blobsm, SM_QW, 128, [128, 128], tag="q_w")
    bias_d = cload_cols(blobsm, SM_BIAS, W2, [128, W2], tag="bias_d")
    sc128 = cload_cols(blobsm, SM_SC, 1, [128, 1], tag="scale128")
    qb = cload_cols(blobsm, SM_QB, 1, [128, 1], tag="q_b2")
    kb = cload_cols(blobsm, SM_KB, 1, [128, 1], tag="k_b2")
    vb = cload_cols(blobsm, SM_VB, 1, [128, 1], tag="v_b2")
    pjb = cload_cols(blobsm, SM_PJB, 1, [128, 1], tag="proj_b2")
    f1b = cload_cols(blobsm, SM_F1B, 4, [128, 4], tag="fc1_b2")
    f2b = cload_cols(blobsm, SM_F2B, 1, [128, 1], tag="fc2_b2")
    n1w = cload_cols(blobsm, SM_N1W, 1, [128, 1], tag="n1w")
    n1b = cload_cols(blobsm, SM_N1B, 1, [128, 1], tag="n1b")
    n2w = cload_cols(blobsm, SM_N2W, 1, [128, 1], tag="n2w")
    n2b = cload_cols(blobsm, SM_N2B, 1, [128, 1], tag="n2b")
    eps24 = cload_cols(blobsm, SM_EPS24, 1, [128, 1], tag="eps24")
    eps6 = cload_cols(blobsm, SM_EPS6, 1, [128, 1], tag="eps6")

    kvw = cload_wblk(0, "kv_w", nblk=2)          # [128,256] bf16
    pjw0 = cload_wblk(2, "proj_w0")
    pjw1 = cload_wblk(3, "proj_w1")
    f1w = cload_wblk(4, "fc1_w", nblk=4)         # [128,512] bf16
    f2ws = [cload_wblk(8 + g, f"fc2_w{g}") for g in range(4)]

    def l2norm_slab(t, n):
        """Per-head l2 normalize columns of a [128, n] channel-major tile."""
        csz = 512
        nchunks = (n + csz - 1) // csz
        for i in range(nchunks):
            lo = i * csz
            m = min(csz, n - lo)
            s = slice(lo, lo + m)
            sq = work.tile([128, csz], F32, tag="sq")
            nc.vector.tensor_mul(sq[:, :m], t[:, s], t[:, s])
            ps = psum.tile([128, csz], F32, tag="mm")
            nc.tensor.matmul(ps[:, :m], e128f[:], sq[:, :m])
            sd = work.tile([128, csz], F32, tag="sd")
            nc.scalar.activation(sd[:, :m], ps[:, :m], AF.Sqrt, bias=eps24[:])
            rn = work.tile([128, csz], F32, tag="rn")
            nc.vector.reciprocal(rn[:, :m], sd[:, :m])
            nc.vector.tensor_mul(t[:, s], t[:, s], rn[:, :m])

    def project(src_t, npix, w_ap, bias_t, out_tile):
        """out = (w.T @ src) + b, channel-major."""
        nchunks = (npix + 511) // 512
        for i in range(nchunks):
            lo = i * 512
            m = min(512, npix - lo)
            s = slice(lo, lo + m)
            ps = psum.tile([128, 512], F32, tag="mm")
            nc.tensor.matmul(ps[:, :m], w_ap, src_t[:, s])
            nc.vector.tensor_scalar_add(out_tile[:, s], ps[:, :m], bias_t[:])

    def restride(flat_t, slab_t, nrows, row0):
        """[128, nrows*192] -> padded slab rows row0.. via SBUF DMA."""
        src = flat_t[:, :nrows * W].rearrange("p (r w) -> p r w", r=nrows)
        dst = slab_t[:, GUARD:GUARD + SSLAB].rearrange(
            "p (r w) -> p r w", r=SHR)[:, row0:row0 + nrows, MD:MD + W]
        nc.sync.dma_start(dst, src)

    delta_dram = io["delta"]

    for b in range(B):
        for st in range(NST):
            # global input offsets for this pass
            hoff = (b * HR + st * SR) * W          # into x0h/x1h (haloed rows)
            toff = (b * RPC + st * SR) * W         # into xt / delta rows

            # ---- slabs ----
            q_s = slabs.tile([128, SNOWN + 2 * GUARD], F32, tag="q_s")
            k0_s = slabs.tile([128, SSLAB + 2 * GUARD], F32, tag="k0_s")
            k1_s = slabs.tile([128, SSLAB + 2 * GUARD], F32, tag="k1_s")
            v0_s = slabs.tile([128, SSLAB + 2 * GUARD], BF16, tag="v0_s")
            v1_s = slabs.tile([128, SSLAB + 2 * GUARD], BF16, tag="v1_s")
            if b == 0 and st == 0:
                # pads/guards stay zero across passes: restrides only write
                # data columns and l2norm maps 0 -> 0 in place
                for t in (q_s, k0_s, k1_s, v0_s, v1_s):
                    nc.gpsimd.memset(t[:], 0.0)

            # ---- x0/x1 -> k/v slabs ----
            for (bq, scoff, k_t, v_t) in ((io["bq0"], SC_X0, k0_s, v0_s),
                                          (io["bq1"], SC_X1, k1_s, v1_s)):
                nt = SNHPIX // 128
                sct = work.tile([128, 32], F32, tag="sct")
                c0 = scoff + hoff // 128
                nc.sync.dma_start(sct[:, :nt], blobsm[:, c0:c0 + nt])
                xu = slabs.tile([128, SNHPIX], BF16, tag="xu")
                for i in range(nt):
                    r = hoff + i * 128
                    tq = post.tile([128, 128], I8, tag="tin8")
                    nc.sync.dma_start(tq[:], bq[r:r + 128, :])
                    xt_ = post.tile([128, 128], BF16, tag="tin")
                    nc.scalar.activation(xt_[:], tq[:], AF.Copy,
                                         scale=sct[:, i:i + 1])
                    pt = psum.tile([128, 128], BF16, tag="ptr16")
                    nc.tensor.matmul(pt[:], xt_[:], eye16[:], is_transpose=True)
                    if i % 2 == 0:
                        nc.vector.tensor_copy(xu[:, i * 128:(i + 1) * 128], pt[:])
                    else:
                        nc.scalar.copy(xu[:, i * 128:(i + 1) * 128], pt[:])
                ku = slabs.tile([128, SNHPIX], F32, tag="ku")
                project(xu, SNHPIX, kvw[:, 0:128], kb, ku)
                vu = slabs.tile([128, SNHPIX], BF16, tag="vu")
                project(xu, SNHPIX, kvw[:, 128:256], vb, vu)
                restride(ku, k_t, SHR, 0)
                restride(vu, v_t, SHR, 0)
                l2norm_slab(k_t[:, GUARD:GUARD + SSLAB], SSLAB)

            # ---- xt -> q slab (+ keep f32 transposed copy for MLP input) ----
            ntx = SNPIX // 128
            sct = work.tile([128, 32], F32, tag="sct")
            cx0 = SC_XT + toff // 128
            nc.sync.dma_start(sct[:, :ntx], blobsm[:, cx0:cx0 + ntx])
            xtu = slabs.tile([128, SNPIX], F32, tag="xtu")
            for i in range(ntx):
                r = toff + i * 128
                tq = post.tile([128, 128], I8, tag="tin8")
                nc.sync.dma_start(tq[:], io["bqt"][r:r + 128, :])
                xt_ = post.tile([128, 128], BF16, tag="tin")
                nc.scalar.activation(xt_[:], tq[:], AF.Copy,
                                     scale=sct[:, i:i + 1])
                pt = psum.tile([128, 128], BF16, tag="ptr16")
                nc.tensor.matmul(pt[:], xt_[:], eye16[:], is_transpose=True)
                if i % 2 == 0:
                    nc.vector.tensor_copy(xtu[:, i * 128:(i + 1) * 128], pt[:])
                else:
                    nc.scalar.copy(xtu[:, i * 128:(i + 1) * 128], pt[:])
            qu = slabs.tile([128, SNPIX], F32, tag="vu")
            project(xtu, SNPIX, qw[:], qb, qu)
            # q slab: own rows only, [128, 12*200] + guards
            src = qu[:].rearrange("p (r w) -> p r w", r=SR)
            dstq = q_s[:, GUARD:GUARD + SNOWN].rearrange(
                "p (r w) -> p r w", r=SR)[:, :, MD:MD + W]
            nc.sync.dma_start(dstq, src)
            l2norm_slab(q_s[:, GUARD:GUARD + SNOWN], SNOWN)

            # ---- attention: 81 shifted passes over 5 chunks ----
            xb_s = slabs.tile([128, SNOWN], F32, tag="xu")
            xf_s = slabs.tile([128, SNOWN], F32, tag="ku")
            for ci in range(NCH):
                oo = ci * CHSZ
                o = OWN0 + oo                 # in k/v slab padded flat coords
                oq = GUARD + oo               # in q slab coords
                qc = q_s[:, oq:oq + CHSZ]
                xbc = xb_s[:, oo:oo + CHSZ]
                xfc = xf_s[:, oo:oo + CHSZ]
                zc = work.tile([128, CHSZ], F32, tag="zc")
                first = True
                for dy in range(-MD, MD + 1):
                    for dx in range(-MD, MD + 1):
                        d = (dy + MD) * WS + (dx + MD)
                        sh_b = o - dy * PW - dx   # k0/v0 at p-d
                        sh_f = o + dy * PW + dx   # k1/v1 at p+d
                        pr0 = dloop.tile([128, CHSZ], F32, tag="pr0")
                        nc.vector.tensor_mul(pr0[:], qc, k0_s[:, sh_b:sh_b + CHSZ])
                        pr1 = dloop.tile([128, CHSZ], F32, tag="pr1")
                        nc.vector.tensor_mul(pr1[:], qc, k1_s[:, sh_f:sh_f + CHSZ])
                        pl = psum.tile([128, CHSZ], F32, tag="mm")
                        nc.tensor.matmul(pl[:], e128f[:], pr0[:], start=True, stop=False)
                        nc.tensor.matmul(pl[:], e128f[:], pr1[:], start=False, stop=True)
                        # a = exp(scale*logit + bias_d); no max-subtraction
                        # needed: |scale*logit| <= 200, safe in fp32.
                        ar = dloop.tile([128, CHSZ], BF16, tag="ar")
                        nc.scalar.activation(ar[:], pl[:], AF.Exp,
                                             bias=bias_d[:, d:d + 1], scale=sc128[:])
                        t0 = dloop.tile([128, CHSZ], BF16, tag="t0")
                        nc.vector.tensor_mul(t0[:], ar[:], v0_s[:, sh_b:sh_b + CHSZ])
                        t1 = dloop.tile([128, CHSZ], BF16, tag="t1")
                        nc.gpsimd.tensor_mul(t1[:], ar[:], v1_s[:, sh_f:sh_f + CHSZ])
                        if first:
                            nc.vector.tensor_copy(zc[:], ar[:])
                            nc.vector.tensor_copy(xbc, t0[:])
                            nc.gpsimd.tensor_copy(xfc, t1[:])
                            first = False
                        else:
                            nc.vector.tensor_add(zc[:], zc[:], ar[:])
                            nc.vector.tensor_add(xbc, xbc, t0[:])
                            nc.gpsimd.tensor_add(xfc, xfc, t1[:])
                rz = work.tile([128, CHSZ], F32, tag="rz")
                nc.vector.reciprocal(rz[:], zc[:])
                nc.vector.tensor_mul(xbc, xbc, rz[:])
                nc.vector.tensor_mul(xfc, xfc, rz[:])

            # repack padded own-window -> unpadded [128, 2304]
            xbu = slabs.tile([128, SNPIX], F32, tag="xbu")
            xfu = slabs.tile([128, SNPIX], F32, tag="xfu")
            for (srct, dstt) in ((xb_s, xbu), (xf_s, xfu)):
                sv = srct[:].rearrange("p (r w) -> p r w", r=SR)[:, :, MD:MD + W]
                dv = dstt[:].rearrange("p (r w) -> p r w", r=SR)
                nc.sync.dma_start(dv, sv)

            # ---- proj + LN1; MLP + LN2; delta = LN1 + LN2 ----
            def layernorm(y_t, w_t, b_t, out_t, m):
                pm = psum.tile([128, 512], F32, tag="mm")
                nc.tensor.matmul(pm[:, :m], j128[:], y_t[:, :m])
                xc = post.tile([128, 512], F32, tag="xc")
                nc.vector.tensor_sub(xc[:, :m], y_t[:, :m], pm[:, :m])
                sq = post.tile([128, 512], F32, tag="lsq")
                nc.vector.tensor_mul(sq[:, :m], xc[:, :m], xc[:, :m])
                pv = psum.tile([128, 512], F32, tag="mm")
                nc.tensor.matmul(pv[:, :m], j128[:], sq[:, :m])
                sd = post.tile([128, 512], F32, tag="lsd")
                nc.scalar.activation(sd[:, :m], pv[:, :m], AF.Sqrt, bias=eps6[:])
                rs = post.tile([128, 512], F32, tag="lrs")
                nc.vector.reciprocal(rs[:, :m], sd[:, :m])
                nc.vector.tensor_mul(xc[:, :m], xc[:, :m], rs[:, :m])
                nc.vector.tensor_scalar(out_t[:, :m], xc[:, :m], w_t[:], b_t[:],
                                        op0=OP.mult, op1=OP.add)

            xa = slabs.tile([128, SNPIX], F32, tag="xa")
            nchp = (SNPIX + 511) // 512
            for ci in range(nchp):
                lo = ci * 512
                m = min(512, SNPIX - lo)
                s = slice(lo, lo + m)
                xb16 = post.tile([128, 512], BF16, tag="xb16")
                nc.vector.tensor_copy(xb16[:, :m], xbu[:, s])
                xf16 = post.tile([128, 512], BF16, tag="xf16")
                nc.scalar.copy(xf16[:, :m], xfu[:, s])
                pp = psum.tile([128, 512], F32, tag="mm")
                nc.tensor.matmul(pp[:, :m], pjw0[:], xb16[:, :m], start=True, stop=False)
                nc.tensor.matmul(pp[:, :m], pjw1[:], xf16[:, :m], start=False, stop=True)
                y = post.tile([128, 512], F32, tag="y")
                nc.vector.tensor_scalar_add(y[:, :m], pp[:, :m], pjb[:])
                ln = post.tile([128, 512], F32, tag="ln")
                layernorm(y, n1w, n1b, ln, m)
                nc.vector.tensor_add(xa[:, s], xtu[:, s], ln[:, :m])

                xa16 = post.tile([128, 512], BF16, tag="xa16")
                nc.vector.tensor_copy(xa16[:, :m], xa[:, s])
                hts = []
                for g in range(4):
                    ph = psum.tile([128, 512], F32, tag="mm")
                    nc.tensor.matmul(ph[:, :m], f1w[:, g * 128:(g + 1) * 128],
                                     xa16[:, :m])
                    ht = post.tile([128, 512], BF16, tag=f"ht{g}")
                    nc.scalar.activation(ht[:, :m], ph[:, :m], AF.Gelu,
                                         bias=f1b[:, g:g + 1])
                    hts.append(ht)
                po = psum.tile([128, 512], F32, tag="mm")
                for g in range(4):
                    nc.tensor.matmul(po[:, :m], f2ws[g][:], hts[g][:, :m],
                                     start=(g == 0), stop=(g == 3))
                y2 = post.tile([128, 512], F32, tag="y2")
                nc.vector.tensor_scalar_add(y2[:, :m], po[:, :m], f2b[:])
                ln2 = post.tile([128, 512], F32, tag="ln2")
                layernorm(y2, n2w, n2b, ln2, m)
                dl = post.tile([128, 512], BF16, tag="oc")
                nc.vector.tensor_add(dl[:, :m], ln[:, :m], ln2[:, :m])

                # transpose back and store this chunk (m is a multiple of 128)
                for i in range(m // 128):
                    pt = psum.tile([128, 128], BF16, tag="ptr16")
                    nc.tensor.matmul(pt[:], dl[:, i * 128:(i + 1) * 128], eye16[:],
                                     is_transpose=True)
                    og = work.tile([128, 128], BF16, tag="otb")
                    if i % 2 == 0:
                        nc.vector.tensor_copy(og[:], pt[:])
                    else:
                        nc.scalar.copy(og[:], pt[:])
                    row = toff + lo + i * 128
                    nc.sync.dma_start(delta_dram[row:row + 128, :], og[:])


_CACHE = {}


def _get_program():
    if "prog" in _CACHE:
        return _CACHE["prog"]
    nc = bacc.Bacc("TRN2", target_bir_lowering=False, debug=False,
                   num_devices=NCORES)
    io = {}
    io["blob16"] = nc.dram_tensor("blob16", [R16, C], BF16,
                                  kind="ExternalInput").ap()
    io["bq0"] = nc.dram_tensor("bq0", [RQH, C], I8,
                               kind="ExternalInput").ap()
    io["bq1"] = nc.dram_tensor("bq1", [RQH, C], I8,
                               kind="ExternalInput").ap()
    io["bqt"] = nc.dram_tensor("bqt", [RQT, C], I8,
                               kind="ExternalInput").ap()
    io["blobsm"] = nc.dram_tensor("blobsm", [128, SMW], F32,
                                  kind="ExternalInput").ap()
    io["constf"] = nc.dram_tensor("constf", [128, CFW], F32,
                                  kind="ExternalInput").ap()
    io["eye16c"] = nc.dram_tensor("eye16c", [128, 128], BF16,
                                  kind="ExternalInput").ap()
    io["delta"] = nc.dram_tensor("delta", [B * NPIX, C], BF16,
                                 kind="ExternalOutput").ap()
    ctx = ExitStack()
    with ctx:
        tc = ctx.enter_context(tile.TileContext(nc, trace_sim=False))
        _trace(ctx, tc, io)
    nc.compile()
    _CACHE["prog"] = nc
    return nc


def _get_runner():
    if "runner" in _CACHE:
        return _CACHE["runner"]
    import jax
    from jax.sharding import Mesh, PartitionSpec, NamedSharding
    from jax.experimental.shard_map import shard_map
    from concourse import bass2jax

    nc = _get_program()
    bass2jax.install_neuronx_cc_hook()
    partition_name = (nc.partition_id_tensor.name
                      if nc.partition_id_tensor else None)

    in_names, out_names, out_avals = [], [], []
    for alloc in nc.m.functions[0].allocations:
        if not isinstance(alloc, mybir.MemoryLocationSet):
            continue
        name = alloc.memorylocations[0].name
        if alloc.kind == "ExternalInput":
            if name != partition_name:
                in_names.append(name)
        elif alloc.kind == "ExternalOutput":
            out_names.append(name)
            out_avals.append(jax.core.ShapedArray(
                tuple(alloc.tensor_shape), mybir.dt.np(alloc.dtype)))
    n_params = len(in_names)
    n_outs = len(out_avals)
    all_in_names = list(in_names) + list(out_names)
    if partition_name is not None:
        all_in_names.append(partition_name)
    donate = tuple(range(n_params, n_params + n_outs))

    def _body(*args):
        operands = list(args)
        if partition_name is not None:
            operands.append(bass2jax.partition_id_tensor())
        outs = bass2jax._bass_exec_p.bind(
            *operands,
            out_avals=tuple(out_avals),
            in_names=tuple(all_in_names),
            out_names=tuple(out_names),
            lowering_input_output_aliases=(),
            sim_require_finite=True,
            sim_require_nnan=True,
            nc=nc,
        )
        return tuple(outs)

    devices = jax.devices()[:NCORES]
    mesh = Mesh(np.asarray(devices), ("core",))
    sh = NamedSharding(mesh, PartitionSpec("core"))
    in_specs = (PartitionSpec("core"),) * (n_params + n_outs)
    out_specs = (PartitionSpec("core"),) * n_outs
    sharded = jax.jit(
        shard_map(_body, mesh=mesh, in_specs=in_specs, out_specs=out_specs,
                  check_rep=False),
        donate_argnums=donate, keep_unused=True,
    )
    zfn = jax.jit(
        lambda: jax.numpy.zeros((NCORES * B * NPIX, C), jax.numpy.bfloat16),
        out_shardings=sh)
    runner = {"jax": jax, "sharded": sharded, "in_names": in_names,
              "out_names": out_names, "sh": sh, "zfn": zfn}
    _CACHE["runner"] = runner
    return runner


def _host_small_blob(q_w, q_b, kv_b, logit_scale, cpb_w1, cpb_b1, cpb_w2,
                     proj_b, norm1_w, norm1_b, fc1_b, fc2_b, norm2_w, norm2_b):
    """Small f32 per-core blob [128, SMW] (input-value dependent)."""
    gy, gx = np.meshgrid(np.arange(WS, dtype=np.float32) * 2.0,
                         np.arange(WS, dtype=np.float32) * 2.0, indexing="ij")
    t = np.stack([gy / (WS - 1) - 1.0, gx / (WS - 1) - 1.0], -1) * 8.0
    t = np.sign(t) * np.log2(np.abs(t) + 1.0) / np.log2(8.0)
    coords = t.reshape(-1, 2)
    hmid = np.maximum(coords @ cpb_w1 + cpb_b1, 0.0)
    bias = 16.0 / (1.0 + np.exp(-(hmid @ cpb_w2)))   # (81, NH)
    head_of_c = (np.arange(128) // HC)
    bias128 = np.ascontiguousarray(bias.T[head_of_c, :]).astype(np.float32)
    scale = np.exp(np.minimum(logit_scale.reshape(NH), np.log(100.0)))

    sm = np.zeros((128, SMW), np.float32)
    sm[:, SM_QW:SM_QW + 128] = q_w
    sm[:, SM_BIAS:SM_BIAS + W2] = bias128
    sm[:, SM_SC] = scale[head_of_c]
    sm[:, SM_QB] = q_b
    sm[:, SM_KB] = kv_b[:128]
    sm[:, SM_VB] = kv_b[128:]
    sm[:, SM_PJB] = proj_b
    sm[:, SM_F1B:SM_F1B + 4] = fc1_b.reshape(4, 128).T
    sm[:, SM_F2B] = fc2_b
    sm[:, SM_N1W] = norm1_w
    sm[:, SM_N1B] = norm1_b
    sm[:, SM_N2W] = norm2_w
    sm[:, SM_N2B] = norm2_b
    sm[:, SM_EPS24] = 1e-24
    sm[:, SM_EPS6] = 1e-6
    return sm


def _static_consts():
    e128 = np.zeros((128, 128), np.float32)
    for h in range(NH):
        e128[h * HC:(h + 1) * HC, h * HC:(h + 1) * HC] = 1.0
    cf = np.empty((128, CFW), np.float32)
    cf[:, CF_E128:CF_E128 + 128] = e128
    cf[:, CF_J128:CF_J128 + 128] = 1.0 / 128.0
    cf[:, CF_EYE:CF_EYE + 128] = np.eye(128, dtype=np.float32)
    eye16 = np.eye(128, dtype=np.float32).astype(NPBF16)
    return cf, eye16


def kernel(x0, x1, xt, q_w, q_b, kv_w, kv_b, logit_scale, cpb_w1, cpb_b1,
           cpb_w2, proj_w, proj_b, norm1_w, norm1_b, fc1_w, fc1_b, fc2_w,
           fc2_b, norm2_w, norm2_b, h, w):
    import os, time
    dbg = os.environ.get("KERNEL_TIMERS")
    tmarks = [("start", time.time())]

    def mark(name):
        if dbg:
            tmarks.append((name, time.time()))

    r = _get_runner()
    jax = r["jax"]
    mark("runner")

    import queue, threading

    x0 = np.asarray(x0, np.float32).reshape(B, H, W, C)
    x1 = np.asarray(x1, np.float32).reshape(B, H, W, C)
    xt = np.asarray(xt, np.float32).reshape(B, H, W, C)

    sh = r["sh"]
    jax_ = jax

    # transfer thread: streams each array to the devices as soon as the
    # (single-cpu) host finishes producing it; the relay transfer is
    # I/O-bound and overlaps the remaining numpy work.
    tq_ = queue.Queue()
    dev_arrs = {}

    def _xfer_worker():
        while True:
            item = tq_.get()
            if item is None:
                return
            name, arr = item
            a = jax_.device_put(arr, sh)
            jax_.block_until_ready(a)
            dev_arrs[name] = a

    # daemon: a prep exception must not leave the process hanging on join
    xfer = threading.Thread(target=_xfer_worker, daemon=True)
    xfer.start()

    # ---- bf16 blob first: cheap to build, gets the transfer going ----
    blob = np.empty((NCORES, R16, C), NPBF16)
    kv_w = np.asarray(kv_w, np.float32)
    proj_w = np.asarray(proj_w, np.float32)
    fc1_w = np.asarray(fc1_w, np.float32)
    fc2_w = np.asarray(fc2_w, np.float32)
    wblk = np.empty((NWB, 128, 128), np.float32)
    wblk[0] = kv_w[:, :128]
    wblk[1] = kv_w[:, 128:]
    wblk[2] = proj_w[0:128]
    wblk[3] = proj_w[128:256]
    for g in range(4):
        wblk[4 + g] = fc1_w[:, g * 128:(g + 1) * 128]
        wblk[8 + g] = fc2_w[g * 128:(g + 1) * 128]
    blob[:, WB0:] = wblk.reshape(NWB * 128, 128).astype(NPBF16)[None]
    tq_.put(("blob16", blob.reshape(NCORES * R16, C)))

    qtmp = np.empty((B, H, W, C), np.float32)

    def _quant(x):
        """x (B,H,W,C) f32 -> (q int8 same shape, s f32 (B,H,W)).

        abs-max via max/-min avoids materializing a 37MB |x| temp; zero
        rows give q=0 regardless of inv, so no epsilon branch is needed.
        """
        a = np.maximum(x.max(-1), -x.min(-1))
        s = a * (1.0 / 127.0)
        inv = 127.0 / np.maximum(a, 1e-30)
        np.multiply(x, inv[..., None], out=qtmp)
        np.rint(qtmp, out=qtmp)
        q = qtmp.astype(np.int8)
        return q, s

    scl = np.zeros((NCORES, 128, SMW - SC_X0), np.float32)

    def _halo_blob(xs, ss, scbase):
        bq = np.empty((NCORES, RQH, C), np.int8)
        bv = bq.reshape(NCORES, B, HR, W, C)
        for ci in range(NCORES):
            r0 = ci * RPC - MD
            r1 = r0 + HR
            lo, hi = max(r0, 0), min(r1, H)
            sslab = np.zeros((B, HR, W), np.float32)
            if lo > r0:
                bv[ci, :, :lo - r0] = 0
            bv[ci, :, lo - r0:hi - r0] = xs[:, lo:hi]
            sslab[:, lo - r0:hi - r0] = ss[:, lo:hi]
            if r1 > hi:
                bv[ci, :, hi - r0:] = 0
            nc_ = RQH // 128
            scl[ci, :, scbase:scbase + nc_] = sslab.reshape(nc_, 128).T
        return bq

    x0q, s0 = _quant(x0)
    tq_.put(("bq0", _halo_blob(x0q, s0, 0).reshape(NCORES * RQH, C)))
    x1q, s1 = _quant(x1)
    tq_.put(("bq1", _halo_blob(x1q, s1, SC_X1 - SC_X0).reshape(NCORES * RQH, C)))
    xtq, st_ = _quant(xt)
    bqt = np.empty((NCORES, RQT, C), np.int8)
    bqt.reshape(NCORES, B, RPC, W, C)[:] = xtq.reshape(
        B, NCORES, RPC, W, C).transpose(1, 0, 2, 3, 4)
    tq_.put(("bqt", bqt.reshape(NCORES * RQT, C)))
    stp = st_.reshape(B, NCORES, RPC, W).transpose(1, 0, 2, 3)
    scl[:, :, SC_XT - SC_X0:] = stp.reshape(
        NCORES, RQT // 128, 128).transpose(0, 2, 1)

    sm1 = _host_small_blob(
        np.asarray(q_w, np.float32), np.asarray(q_b, np.float32),
        np.asarray(kv_b, np.float32), np.asarray(logit_scale, np.float32),
        np.asarray(cpb_w1, np.float32), np.asarray(cpb_b1, np.float32),
        np.asarray(cpb_w2, np.float32), np.asarray(proj_b, np.float32),
        np.asarray(norm1_w, np.float32), np.asarray(norm1_b, np.float32),
        np.asarray(fc1_b, np.float32), np.asarray(fc2_b, np.float32),
        np.asarray(norm2_w, np.float32), np.asarray(norm2_b, np.float32))
    smg = np.empty((NCORES, 128, SMW), np.float32)
    smg[:, :, :SC_X0] = sm1[:, :SC_X0]
    smg[:, :, SC_X0:] = scl
    tq_.put(("blobsm", smg.reshape(NCORES * 128, SMW)))
    tq_.put(None)
    mark("host_prep")

    if "constf_dev" not in _CACHE:
        cf, eye16 = _static_consts()
        _CACHE["constf_dev"] = jax.device_put(
            np.broadcast_to(cf, (NCORES, 128, CFW)).reshape(-1, CFW), sh)
        _CACHE["eye16c_dev"] = jax.device_put(
            np.broadcast_to(eye16, (NCORES, 128, 128)).reshape(-1, 128), sh)
    zeros_dev = _CACHE.pop("zeros_dev", None)
    if zeros_dev is None or getattr(zeros_dev, "is_deleted", lambda: False)():
        zeros_dev = r["zfn"]()

    xfer.join()
    mark("puts")

    byname = dict(dev_arrs)
    byname["constf"] = _CACHE["constf_dev"]
    byname["eye16c"] = _CACHE["eye16c_dev"]
    args = [byname[nm] for nm in r["in_names"]]
    out_arrs = r["sharded"](*args, zeros_dev)
    _CACHE["zeros_dev"] = r["zfn"]()   # async; ready for the next call
    if dbg:
        jax.block_until_ready(out_arrs)
    mark("exec")

    delta = np.asarray(out_arrs[0])    # [8*9216, 128] bf16
    mark("fetch")
    dv = delta.reshape(NCORES, B, RPC, W, C).astype(np.float32)
    out = np.empty((B, H, W, C), np.float32)
    ov = out.reshape(B, NCORES, RPC, W, C)
    ov[:] = dv.transpose(1, 0, 2, 3, 4)
    out += xt
    mark("post")
    if dbg:
        import sys as _sys
        parts = " ".join(f"{n}={tmarks[i+1][1]-tmarks[i][1]:.3f}"
                         for i, (n, _) in enumerate(tmarks[1:]))
        print(f"[kernel timers] {parts}", file=_sys.stderr, flush=True)
    return out.reshape(B, H * W, C)
